# revision 5
# baseline (speedup 1.0000x reference)
"""Causal flash attention (B=2, H=16, S=2048, D=64, fp32) on 8 TRN2 NeuronCores.

Strategy: shard batch*heads (32) across 8 cores -> 4 heads/core, processed as
2 head-pairs packed into the 128 SBUF partitions (d=64 each). Per head,
transposed scores S^T[k, q] = K Q^T on PE (fp16 inputs, fp32 PSUM), exp on ACT
(softmax scale folded in, fp16 out), causal mask post-exp as a multiplicative
0/1 fp16 mask on diagonal tiles (DVE), then PV on PE with a ones column
appended to V so the denominator falls out of the same matmul. Output leaves
transposed ([d+1, q] per head, fp16); host divides by the denominator row and
transposes back.

ACT (exp at 1 elem/lane/cycle, 1.2 GHz) is the bottleneck: ~69.6K exp columns
per core is a 58us floor. This version improves on the 93.5us baseline by:
  - diagonal trim: the fully-invalid [128k x 128q] quarter of each q-block's
    diagonal k-tile pair is never computed (5.5% less exp/QK/PV work). Tiles
    are 256 q wide except the last diagonal k-tile of each q-block (128).
  - exp groups of [128, 1536] fp32 (3 PSUM banks, 2 buffered) packed from a
    flat tile stream that crosses q-block and pair boundaries: 48 ACT
    instructions instead of 72 amortizes the ~192-cycle/instr overhead.
  - both heads' PV accumulators share one PSUM bank ([65, 512] = pvA|pvB):
    the head-A kt=0 matmul's start=True clears the whole bank's has_written
    bits, then head-B kt=0 runs start=False and overwrites (bits clear).
    6 score banks + 2 pv banks = all 8, double buffered everywhere.
  - ascending q-block order: the first compute needs only k[0:256]/q[0:256]
    (small first DMA chunks land fast) and output stores spread out instead
    of bunching at the tail (the sync-ring DIRECT2D dispatch is ~800ns each).
  - a dummy 8-col exp issued at t=0 pulls the ~2.7us ACT table load off the
    critical path, overlapping it with the input DMAs.
  - v DRAM layout [128, pr, NKT, h, D+1] gives 1 descriptor/partition per
    chunk (the old layout generated 8K 130-byte descriptors).

PSUM bank rules (hardware-faulting otherwise): the two concurrently executing
row-group matmuls of a QK pair must write DIFFERENT banks, and no matmul
output may cross a bank boundary. Group layout: head-A chunks at col a_j
(fulls 256-aligned first, then 128-wide halves), head-B at 768 + a_j; the
768-col shift guarantees distinct banks for every (A_j, B_j) pair and
256/128-aligned chunks never cross a 512-col bank edge.
"""

import numpy as np

B, H, S, D = 2, 16, 2048, 64
BH = B * H
NCORES = 8
HPC = BH // NCORES  # heads per core
SCALE = 0.125
W = 256             # q-block width
TK = 128            # k-tile height
NKT = S // TK       # 16 k-tiles
NQB = S // W        # 8 q-blocks
HG = 768            # per-head max cols per exp group (B side offset)

# If True, split each PV matmul into two 64-row (kv) halves on disjoint PE
# row groups so they can run concurrently and their LDWEIGHTS pull ahead.
PV_SPLIT = False
# Groups routed to the DVE via the Schraudolph exp2 bit-trick (bf16 pattern
# built directly as int16 = s * A16 + B16): frees ~1.4us of ACT per group at
# ~1.7us of otherwise-idle DVE. Max rel err of the approx is ~3%; applied to
# ~17% of scores the output Frobenius error stays ~5e-3 (gate 2e-2).
DVE_GROUPS = frozenset((5, 10, 15, 20, 25, 30, 35, 40))
A16 = (128.0 / 0.6931471805599453) * SCALE   # 2^7/ln2 * softmax scale
B16 = 16256.0 - 366392.0 / 65536.0           # 127*2^7 - C/2^16

_CACHE = {}


def _tile_stream():
    """Flat (pr, qb, kt, w, q0, masked) stream, ascending qb within pair."""
    tiles = []
    for pr in range(2):
        for qb in range(NQB):
            for kt in range(2 * qb + 1):
                tiles.append((pr, qb, kt, W, qb * W, kt == 2 * qb))
            tiles.append((pr, qb, 2 * qb + 1, 128, qb * W + 128, True))
    return tiles


def _pack_groups():
    """Pack the tile stream into groups of <= HG head-cols; within a group,
    full-width tiles first (256-aligned) then halves (128-aligned) so no
    matmul output crosses a PSUM bank boundary."""
    groups, cur, width = [], [], 0
    for t in _tile_stream():
        if width + t[3] > HG:
            groups.append(cur)
            cur, width = [], 0
        cur.append(t)
        width += t[3]
    groups.append(cur)
    packed = []
    for g in groups:
        fulls = [t for t in g if t[3] == W]
        halves = [t for t in g if t[3] != W]
        off, placed = 0, []
        for t in fulls + halves:
            placed.append((t, off))
            off += t[3]
        packed.append((placed, off))  # (list of (tile, a_off), group width)
    return packed


def _build_nc():
    import concourse.bass as bass  # noqa: F401
    import concourse.mybir as mybir
    import concourse.tile as tile
    from concourse import bacc

    f32 = mybir.dt.float32
    f16 = mybir.dt.float16
    bf16 = mybir.dt.bfloat16
    i16 = mybir.dt.int16
    EXP = mybir.ActivationFunctionType.Exp

    nc = bacc.Bacc("TRN2", target_bir_lowering=False, debug=False, num_devices=NCORES)

    # Host-swizzled layouts so every DMA reads contiguous rows.
    kt_d = nc.dram_tensor("ktp", [128, 2, S], f16, kind="ExternalInput").ap()
    qt_d = nc.dram_tensor("qtp", [128, 2, S], f16, kind="ExternalInput").ap()
    v_d = nc.dram_tensor("vxp", [128, 2, NKT, 2, D + 1], bf16,
                         kind="ExternalInput").ap()
    o_d = nc.dram_tensor("outT", [HPC, D + 1, S], f16, kind="ExternalOutput").ap()

    groups = _pack_groups()

    with tile.TileContext(nc) as tc:
        const_pool = tc.alloc_tile_pool(name="const", bufs=1)
        kq_pool = tc.alloc_tile_pool(name="kq", bufs=1)
        vx_pool = tc.alloc_tile_pool(name="vx", bufs=1)
        p_pool = tc.alloc_tile_pool(name="p", bufs=6)
        o_pool = tc.alloc_tile_pool(name="o", bufs=4)
        ps_pool = tc.alloc_tile_pool(name="ps", bufs=2, space="PSUM")
        pv_pool = tc.alloc_tile_pool(name="pv", bufs=2, space="PSUM")

        # Dummy exp at t=0: walrus attaches the ACT table load to the first
        # activation in program order, so this overlaps it with input DMAs.
        scratch_in = const_pool.tile([128, 8], f32, name="dummy_in")
        scratch_out = const_pool.tile([128, 8], f16, name="dummy_out")
        nc.vector.memset(scratch_in[:], 0.0)
        nc.scalar.activation(scratch_out[:], scratch_in[:], EXP, scale=SCALE)

        # Causal mask for diagonal tiles: maskA[x, y] = 1 if y >= x.
        # Full diagonal tiles use all 256 cols; half tiles use cols 0:128.
        # Emitted before the v dma_starts so the gpsimd engine runs it first
        # (~240ns) without delaying SWDGE descriptor generation much.
        maskA = const_pool.tile([128, W], bf16, name="maskA")
        nc.gpsimd.memset(maskA[:], 1.0)
        nc.gpsimd.affine_select(
            out=maskA[:], in_=maskA[:],
            compare_op=mybir.AluOpType.is_ge,
            fill=0.0, base=0,
            pattern=[[1, W]], channel_multiplier=-1,
        )

        ktpp = [kq_pool.tile([128, S], f16, name=f"ktp{r}", tag=f"ktp{r}")
                for r in range(2)]
        qtpp = [kq_pool.tile([128, S], f16, name=f"qtp{r}", tag=f"qtp{r}")
                for r in range(2)]
        vxtp = [vx_pool.tile([128, NKT, 2, D + 1], bf16, name=f"vxt{r}",
                             tag=f"vxt{r}") for r in range(2)]

        # Input loads, first-needed pieces first (ascending qb: k-tiles and
        # q-blocks both ascend). k on the sync HWDGE ring, q on the scalar
        # ring, v on the gpsimd SWDGE ring (all parallel).
        nc.sync.dma_start(ktpp[0][:, 0:256], kt_d[:, 0, 0:256])
        nc.scalar.dma_start(qtpp[0][:, 0:256], qt_d[:, 0, 0:256])
        nc.gpsimd.dma_start(vxtp[0][:, 0:2], v_d[:, 0, 0:2])
        nc.sync.dma_start(ktpp[0][:, 256:768], kt_d[:, 0, 256:768])
        nc.gpsimd.dma_start(vxtp[0][:, 2:6], v_d[:, 0, 2:6])
        nc.scalar.dma_start(qtpp[0][:, 256:768], qt_d[:, 0, 256:768])
        nc.sync.dma_start(ktpp[0][:, 768:1280], kt_d[:, 0, 768:1280])
        nc.gpsimd.dma_start(vxtp[0][:, 6:10], v_d[:, 0, 6:10])
        nc.sync.dma_start(ktpp[0][:, 1280:S], kt_d[:, 0, 1280:S])
        nc.scalar.dma_start(qtpp[0][:, 768:S], qt_d[:, 0, 768:S])
        nc.gpsimd.dma_start(vxtp[0][:, 10:NKT], v_d[:, 0, 10:NKT])
        nc.sync.dma_start(ktpp[1][:], kt_d[:, 1, :])
        nc.scalar.dma_start(qtpp[1][:], qt_d[:, 1, :])
        nc.gpsimd.dma_start(vxtp[1][:], v_d[:, 1])

        state = {"pending": None, "store_count": 0}
        pv_tiles = {}

        def flush_pending():
            pending = state["pending"]
            if pending is None:
                return
            placed, p = pending
            ordered = ([t for t in placed if not t[0][5]]
                       + [t for t in placed if t[0][5]])
            for (pr, qb, kt, w, q0, masked), a in ordered:
                if kt == 0:
                    pv_tiles[(pr, qb)] = pv_pool.tile(
                        [D + 1, 2 * W], f32, tag="pv", name="pv")
                pv = pv_tiles[(pr, qb)]
                vxt = vxtp[pr]
                qoff = q0 - qb * W  # 0 for full tiles, 128 for halves
                last = kt == 2 * qb + 1
                if PV_SPLIT:
                    for h in range(2):
                        for r in range(2):
                            nc.tensor.matmul(
                                pv[:, h * W + qoff:h * W + qoff + w],
                                vxt[64 * r:64 * r + 64, kt, h, :],
                                p[:, HG * h + a:HG * h + a + w],
                                start=(kt == 0 and h == 0 and r == 0),
                                stop=last, skip_group_check=True,
                            )
                else:
                    for h in range(2):
                        nc.tensor.matmul(
                            pv[:, h * W + qoff:h * W + qoff + w],
                            vxt[:, kt, h, :],
                            p[:, HG * h + a:HG * h + a + w],
                            start=(kt == 0 and h == 0),
                            stop=last, skip_group_check=True,
                        )
                if last:  # q-block complete: copy out + store
                    o = o_pool.tile([D + 1, 2 * W], f16, tag="o", name="o")
                    final = (pr, qb) == (1, NQB - 1)
                    for h in range(2):
                        nc.vector.tensor_copy(
                            o[:, h * W:(h + 1) * W], pv[:, h * W:(h + 1) * W])
                        if final:  # overlap store dispatch with the 2nd copy
                            ring = (nc.sync, nc.scalar)[h]
                            ring.dma_start(
                                o_d[2 * pr + h, :, qb * W:(qb + 1) * W],
                                o[:, h * W:(h + 1) * W])
                    if not final:
                        for h in range(2):
                            ring = (nc.sync, nc.scalar)[state["store_count"] % 2]
                            state["store_count"] += 1
                            ring.dma_start(
                                o_d[2 * pr + h, :, qb * W:(qb + 1) * W],
                                o[:, h * W:(h + 1) * W])
                    del pv_tiles[(pr, qb)]
            state["pending"] = None

        for gi, (placed, width) in enumerate(groups):
            sG = ps_pool.tile([128, 2 * HG], f32, tag="sG", name="sG")
            for (pr, qb, kt, w, q0, masked), a in placed:
                ktp, qtp = ktpp[pr], qtpp[pr]
                for h in range(2):
                    nc.tensor.matmul(
                        sG[:, HG * h + a:HG * h + a + w],
                        ktp[64 * h:64 * h + 64, kt * TK:(kt + 1) * TK],
                        qtp[64 * h:64 * h + 64, q0:q0 + w],
                        start=True, stop=True,
                    )
            p = p_pool.tile([128, 2 * HG], bf16, tag="p", name="p")
            span = HG + width
            if gi in DVE_GROUPS:
                nc.vector.tensor_scalar(
                    p[:, :span].bitcast(i16), sG[:, :span], A16, B16,
                    op0=mybir.AluOpType.mult, op1=mybir.AluOpType.add)
            else:
                nc.scalar.activation(p[:, :span], sG[:, :span], EXP, scale=SCALE)
            for (pr, qb, kt, w, q0, masked), a in placed:
                if not masked:
                    continue
                for h in range(2):
                    off = HG * h + a
                    nc.vector.tensor_mul(
                        p[:, off:off + w], p[:, off:off + w], maskA[:, 0:w])
            flush_pending()
            state["pending"] = (placed, p)
        flush_pending()

        pv_pool.release()
        ps_pool.release()
        o_pool.release()
        p_pool.release()
        vx_pool.release()
        kq_pool.release()
        const_pool.release()

    nc.compile()
    return nc


def _get_nc():
    if "nc" not in _CACHE:
        _CACHE["nc"] = _build_nc()
    return _CACHE["nc"]


def _prep_inputs(q, k, v):
    qf = np.ascontiguousarray(np.asarray(q, dtype=np.float32)).reshape(BH, S, D)
    kf = np.ascontiguousarray(np.asarray(k, dtype=np.float32)).reshape(BH, S, D)
    vf = np.ascontiguousarray(np.asarray(v, dtype=np.float32)).reshape(BH, S, D)
    import ml_dtypes
    vx = np.empty((BH, S, D + 1), ml_dtypes.bfloat16)
    vx[:, :, :D] = vf
    vx[:, :, D] = 1.0
    qt = qf.transpose(0, 2, 1).astype(np.float16)  # [BH, D, S]
    kt = kf.transpose(0, 2, 1).astype(np.float16)
    in_maps = []
    for c in range(NCORES):
        sl = slice(HPC * c, HPC * (c + 1))
        # [128, 2, S]: partition = (head-in-pair, d), middle = pair index
        ktp = kt[sl].reshape(2, 128, S).transpose(1, 0, 2)
        qtp = qt[sl].reshape(2, 128, S).transpose(1, 0, 2)
        # [128, 2, NKT, 2, D+1]: partition = kv offset within k-tile,
        # then (pair, k-tile, head-in-pair, d+1) so per-pair chunk loads
        # are contiguous per partition row.
        vxp = vx[sl].reshape(2, 2, NKT, TK, D + 1).transpose(3, 0, 2, 1, 4)
        in_maps.append({
            "ktp": np.ascontiguousarray(ktp),
            "qtp": np.ascontiguousarray(qtp),
            "vxp": np.ascontiguousarray(vxp),
        })
    return in_maps


def _postprocess(results):
    out = np.empty((B, H, S, D), np.float32)
    for c in range(NCORES):
        ot = results[c]["outT"].astype(np.float32)  # [HPC, D+1, S]
        o = (ot[:, :D, :] / ot[:, D:D + 1, :]).transpose(0, 2, 1)  # [HPC, S, D]
        for i in range(HPC):
            bh = HPC * c + i
            out[bh // H, bh % H] = o[i]
    return out


def run(q, k, v, trace=False):
    from concourse.bass_utils import run_bass_kernel_spmd

    nc = _get_nc()
    in_maps = _prep_inputs(q, k, v)
    res = run_bass_kernel_spmd(
        nc, in_maps, core_ids=list(range(NCORES)), trace=trace
    )
    return _postprocess(res.results), res


def kernel(q, k, v):
    out, _ = run(q, k, v, trace=False)
    return out
